# revision 2
# baseline (speedup 1.0000x reference)
"""Trainium2 Bass kernel for nn_Node_GCN: out[n] = f(x[n]) + edge[n]^T @ g(cat(x,x)[n]).

Sharding: data-parallel over the batch dim N=8, one batch per NeuronCore.

v2: edge is carried in fp8(e4m3) — halves the dominant HBM stream vs fp16 —
and gx is quantized to fp8 so the 2048x2048x128 edge contraction runs in
MatmulPerfMode.DoubleRow (2 K-tiles per pass, 0.5 cycles/row). Quantization
error lands at rel_l2 ~1.5e-3 (vs 2e-2 budget): edge in [0,1) keeps e4m3's
ulp at 2^-4 scale and the 2048-term reduction averages the noise down.

Edge is host-reordered partition-major so the whole 4MB stream is 4 DMAs of
128 descriptors x 8KB contiguous (vs 16 DMAs x 128 x 4KB in v1): fewer
serial DMA_DIRECT2D posts on the Sync queue, and each DMA d carries sender
pairs 2d,2d+1 so pair-major matmul consumption follows arrival order.
Small tensors (xT, weights, biases, outT store) ride the GpSimd queue so
they never sit behind the edge stream.

The device computes outT[n] = [h, j] fp16; the host transposes to [j, h]
and widens to fp32 while unsharding.
"""

import numpy as np

D_IN = 64
D_HID = 128
M = 2048          # nodes per batch
N_BATCH = 8
NCORES = 8

NT = M // 128     # 16 sender k-tiles
NPAIR = NT // 2   # 8 DoubleRow pairs
NDMA = 4          # edge DMAs, 2 pairs each
NCH = M // 512    # 4 output chunks of 512

# fp16 weights blob [128, WB_W]; rows 64:128 duplicate rows 0:64 for the
# K=64 matmuls whose rhs lives on partitions 64:128 (xT packed [128, 1024]).
_W_FW1 = 0          # f_w1 [64, 64]
_W_FW2 = 64         # f_w2 [64, 128]
_W_WG1 = 192        # wg1  [64, 128]  (= g_w1[:64] + g_w1[64:])
_W_GW2 = 320        # g_w2 [128, 128]
WB_W = 448

# fp32 bias blob [128, BB_W]
_B_GB2 = 0          # g_b2 broadcast rows, tiled 4x along free dim [128, 512]
_B_F1 = 512         # f_b1 [64, 1]
_B_G1 = 513         # g_b1 [128, 1]
_B_F2 = 514         # f_b2 [128, 1]
BB_W = 515

_NC_CACHE = {}


def _build():
    import concourse.bacc as bacc
    import concourse.mybir as mybir
    from concourse.tile import TileContext
    from concourse.bass import ts

    f32 = mybir.dt.float32
    f16 = mybir.dt.float16
    f8 = mybir.dt.float8e4
    AF = mybir.ActivationFunctionType
    DR = mybir.MatmulPerfMode.DoubleRow

    nc = bacc.Bacc()
    xT_d = nc.declare_dram_parameter("xT", [128, M // 2], f16, isOutput=False)
    # partition-major fp8 edge: [p, d, kk, t, j] flattened to [128, 32768]
    edge_d = nc.declare_dram_parameter("edge", [128, NDMA * 2 * 2 * M], f8, isOutput=False)
    wb_d = nc.declare_dram_parameter("wb", [128, WB_W], f16, isOutput=False)
    bb_d = nc.declare_dram_parameter("bb", [128, BB_W], f32, isOutput=False)
    outT_d = nc.declare_dram_parameter("outT", [D_HID, M], f16, isOutput=True)

    with TileContext(nc) as tc:
        with (
            tc.tile_pool(name="const", bufs=1) as cpool,
            tc.tile_pool(name="acts", bufs=1) as apool,
            tc.tile_pool(name="edgep", bufs=NDMA) as epool,
            tc.tile_pool(name="pout", bufs=1, space="PSUM") as pout_pool,
            tc.tile_pool(name="pg", bufs=2, space="PSUM") as pg_pool,
            tc.tile_pool(name="pwork", bufs=2, space="PSUM") as pwork_pool,
        ):
            wb = cpool.tile([128, WB_W], f16, name="wb")
            bb = cpool.tile([128, BB_W], f32, name="bb")
            xT = cpool.tile([128, M // 2], f16, name="xT")

            # edge tiles [p, kk, t, j]: DMA d carries sender pairs 2d, 2d+1
            eds = [epool.tile([128, 2, 2, M], f8, tag="e", name=f"ed{d}") for d in range(NDMA)]
            for d in range(NDMA):
                nc.sync.dma_start(out=eds[d], in_=edge_d[:, ts(d, 2 * 2 * M)])

            # small tensors on the GpSimd queue, off the edge stream
            nc.gpsimd.dma_start(out=xT[:, 0:512], in_=xT_d[:, 0:512])
            nc.gpsimd.dma_start(out=wb, in_=wb_d[:])
            nc.gpsimd.dma_start(out=bb, in_=bb_d[:])
            nc.gpsimd.dma_start(out=xT[:, 512:1024], in_=xT_d[:, 512:1024])

            w_g2 = wb[0:128, _W_GW2:_W_GW2 + 128]
            w_f2 = wb[0:64, _W_FW2:_W_FW2 + 128]
            gb2b4 = bb[0:128, _B_GB2:_B_GB2 + 512]
            b_f1 = bb[0:64, _B_F1:_B_F1 + 1]
            b_g1 = bb[0:128, _B_G1:_B_G1 + 1]
            b_f2 = bb[0:128, _B_F2:_B_F2 + 1]

            h1f = apool.tile([D_IN, M], f16, name="h1f")
            h1g = apool.tile([D_HID, M], f16, name="h1g")
            gx = apool.tile([128, NT, 128], f8, name="gx")  # tile i: [t, h] node-major
            outT = apool.tile([128, M], f16, name="outT")
            pout = pout_pool.tile([128, M], f32, name="pout")

            # warm the ACT function table during the preamble (hoists the lazy
            # ~1.3us ACT_TABLE_LOAD off the h1g critical path)
            warm = apool.tile([1, 1], f32, name="warm")
            nc.scalar.activation(warm, bb[0:1, 0:1], AF.Relu, bias=0.0)

            # token-chunk c of h1g/h1f reads xT[64a : 64a+64, 512c2 : 512c2+512]
            # with (a, c2) = (c % 2, c // 2): tokens 1024a + 512c2 .. +512.
            def tok_slice(c):
                a, c2 = c % 2, c // 2
                return slice(64 * a, 64 * a + 64), ts(c2, 512)

            def h1g_chunk(c):
                rows, cols = tok_slice(c)
                w_g1 = wb[rows, _W_WG1:_W_WG1 + 128]
                psg = pg_pool.tile([128, 512], f32, tag="g", name="psg")
                nc.tensor.matmul(psg, w_g1, xT[rows, cols], start=True, stop=True)
                nc.scalar.activation(h1g[:, ts(c, 512)], psg, AF.Relu, bias=b_g1)

            def h1f_chunk(c):
                rows, cols = tok_slice(c)
                w_f1 = wb[rows, _W_FW1:_W_FW1 + 64]
                psf = pwork_pool.tile([64, 512], f32, tag="w", name="psf")
                nc.tensor.matmul(psf, w_f1, xT[rows, cols], start=True, stop=True)
                nc.scalar.activation(h1f[:, ts(c, 512)], psf, AF.Relu, bias=b_f1)

            def gx_chunk(c):
                # gx tiles 4c..4c+3 (node-major [t, h]) batched: 4 matmuls into
                # one PSUM bank, one DVE bias-add with fp8 downcast on write
                psx = pwork_pool.tile([128, 512], f32, tag="w", name="psx")
                for k in range(4):
                    i = 4 * c + k
                    nc.tensor.matmul(
                        psx[:, ts(k, 128)], h1g[:, ts(i, 128)], w_g2,
                        start=True, stop=True,
                    )
                nc.vector.tensor_add(gx[:, 4 * c:4 * c + 4, :], psx, gb2b4)

            # ---- schedule ----
            # token-chunk order 0,1,2,3: chunk c unlocks gx tiles 4c..4c+3 =
            # sender pairs 2c, 2c+1 = exactly edge DMA d=c's consumers.
            h1g_chunk(0)
            gx_chunk(0)
            # self-dynamics opens each output chunk's PSUM accumulation group
            for c in range(NCH):
                h1f_chunk(c)
            for c in range(NCH):
                nc.tensor.matmul(
                    pout[:, ts(c, 512)], w_f2, h1f[:, ts(c, 512)],
                    start=True, stop=False,
                )

            def edge_pairs(d):
                for kk in range(2):
                    pair = 2 * d + kk
                    lhsT = gx[:, 2 * pair:2 * pair + 2, :]
                    for c in range(NCH):
                        nc.tensor.matmul(
                            pout[:, ts(c, 512)], lhsT, eds[d][:, kk, :, ts(c, 512)],
                            start=False, stop=(pair == NPAIR - 1 and c == NCH - 1),
                            perf_mode=DR,
                        )

            edge_pairs(0)
            h1g_chunk(1)
            gx_chunk(1)
            edge_pairs(1)
            h1g_chunk(2)
            gx_chunk(2)
            edge_pairs(2)
            h1g_chunk(3)
            gx_chunk(3)
            edge_pairs(3)

            # tail: per-chunk bias-add copy (alternating ACT/DVE), single store
            for c in range(NCH):
                src = pout[:, ts(c, 512)]
                dst = outT[:, ts(c, 512)]
                if c % 2 == 0:
                    nc.scalar.activation(dst, src, AF.Identity, bias=b_f2)
                else:
                    nc.vector.tensor_scalar_add(dst, src, b_f2)
            nc.gpsimd.dma_start(out=outT_d[:], in_=outT)
    nc.compile()
    return nc


def _get_nc():
    if "nc" not in _NC_CACHE:
        _NC_CACHE["nc"] = _build()
    return _NC_CACHE["nc"]


def _prep_in_maps(inputs):
    import ml_dtypes

    x = np.asarray(inputs["x"], dtype=np.float32)
    edge = np.asarray(inputs["edge"], dtype=np.float32)
    f_w1 = np.asarray(inputs["f_w1"], dtype=np.float32)
    f_b1 = np.asarray(inputs["f_b1"], dtype=np.float32)
    f_w2 = np.asarray(inputs["f_w2"], dtype=np.float32)
    f_b2 = np.asarray(inputs["f_b2"], dtype=np.float32)
    g_w1 = np.asarray(inputs["g_w1"], dtype=np.float32)
    g_b1 = np.asarray(inputs["g_b1"], dtype=np.float32)
    g_w2 = np.asarray(inputs["g_w2"], dtype=np.float32)
    g_b2 = np.asarray(inputs["g_b2"], dtype=np.float32)

    # cat(x, x) @ g_w1 == x @ (g_w1[:64] + g_w1[64:])
    wg1 = g_w1[:D_IN] + g_w1[D_IN:]

    wb = np.zeros((128, WB_W), dtype=np.float16)
    for r in (slice(0, 64), slice(64, 128)):  # duplicate for partition-64 rhs
        wb[r, _W_FW1:_W_FW1 + 64] = f_w1.astype(np.float16)
        wb[r, _W_FW2:_W_FW2 + 128] = f_w2.astype(np.float16)
        wb[r, _W_WG1:_W_WG1 + 128] = wg1.astype(np.float16)
    wb[0:128, _W_GW2:_W_GW2 + 128] = g_w2.astype(np.float16)

    bb = np.zeros((128, BB_W), dtype=np.float32)
    bb[0:128, _B_GB2:_B_GB2 + 512] = np.tile(g_b2[None, :], (128, 4))
    bb[0:64, _B_F1] = f_b1
    bb[0:128, _B_G1] = g_b1
    bb[0:128, _B_F2] = f_b2

    # x[n].T packed [128, 1024]: xT2[64a + k, t] = x[n, 1024a + t, k]
    xT = np.transpose(x, (0, 2, 1)).astype(np.float16)       # [8, 64, 2048]
    xT2 = np.concatenate([xT[:, :, :1024], xT[:, :, 1024:]], axis=1)  # [8, 128, 1024]
    xT2 = np.ascontiguousarray(xT2)

    # edge fp8 partition-major: edge_r[n][p, d, kk, t, j] = edge8[n, 128*(4d+2kk+t)+p, j]
    edge8 = edge.astype(ml_dtypes.float8_e4m3)               # [8, 2048, 2048]
    edge_r = np.ascontiguousarray(
        edge8.reshape(N_BATCH, NDMA, 2, 2, 128, M).transpose(0, 4, 1, 2, 3, 5)
    ).reshape(N_BATCH, 128, NDMA * 2 * 2 * M)

    in_maps = [
        {
            "xT": xT2[n],
            "edge": edge_r[n],
            "wb": wb,
            "bb": bb,
        }
        for n in range(N_BATCH)
    ]
    return in_maps


def run(inputs, trace=False, **kw):
    """Run on 8 cores; returns (out [8, 2048, 128] fp32, BassKernelResults)."""
    from concourse.bass_utils import run_bass_kernel_spmd

    nc = _get_nc()
    in_maps = _prep_in_maps(inputs)
    res = run_bass_kernel_spmd(nc, in_maps, list(range(NCORES)), trace=trace, **kw)
    outT = np.stack([np.asarray(res.results[n]["outT"]) for n in range(N_BATCH)])
    out = np.ascontiguousarray(np.transpose(outT, (0, 2, 1)))  # [8, 2048, 128]
    return out.astype(np.float32), res


def kernel(**inputs):
    out, _ = run(inputs, trace=False)
    return out


# revision 3
# speedup vs baseline: 1.0814x; 1.0814x over previous
"""Trainium2 Bass kernel for nn_Node_GCN: out[n] = f(x[n]) + edge[n]^T @ g(cat(x,x)[n]).

Sharding: data-parallel over the batch dim N=8, one batch per NeuronCore.

v3: edge is carried in fp8(e4m3) — halves the dominant HBM stream vs fp16 —
and gx is quantized to fp8 so the 2048x2048x128 edge contraction runs in
MatmulPerfMode.DoubleRow (2 K-tiles per matmul, 216ns/512-col steady-state).
Quantization error lands at rel_l2 ~1.8e-3 (vs 2e-2 budget): edge in [0,1)
keeps e4m3's ulp at 2^-4 scale and the 2048-term reduction averages the
noise down.

All input DMAs ride ONE queue (Sync) ordered small-first — xT+wb packed as
a single fp16 blob, then biases, then 8 pair-granular edge DMAs — so the
MLP inputs land in ~2us and the edge stream follows back-to-back. Edge is
host-reordered partition-major: each pair DMA is 128 descriptors x 4KB
contiguous. The outT store is split in two posts on the idle GpSimd queue,
overlapping the second half's bias-add with the first half's writeback.

The device computes outT[n] = [h, j] fp16; the host transposes to [j, h]
and widens to fp32 while unsharding.
"""

import numpy as np

D_IN = 64
D_HID = 128
M = 2048          # nodes per batch
N_BATCH = 8
NCORES = 8

NT = M // 128     # 16 sender k-tiles
NPAIR = NT // 2   # 8 DoubleRow pairs = 8 edge DMAs
NCH = M // 512    # 4 output chunks of 512

# fp16 blob [128, FB_W]: xT [128, 1024] then weights
_F_XT = 0
_W_FW1 = 1024       # f_w1 [64, 64]
_W_FW2 = 1088       # f_w2 [64, 128]
_W_WG1 = 1216       # wg1  [64, 128]  (= g_w1[:64] + g_w1[64:])
_W_GW2 = 1344       # g_w2 [128, 128]
FB_W = 1472

# fp32 bias blob [128, BB_W]
_B_GB2 = 0          # g_b2 broadcast rows, tiled 4x along free dim [128, 512]
_B_F1 = 512         # f_b1 [64, 1]
_B_G1 = 513         # g_b1 [128, 1]
_B_F2 = 514         # f_b2 [128, 1]
BB_W = 515

_NC_CACHE = {}


def _build():
    import concourse.bacc as bacc
    import concourse.mybir as mybir
    from concourse.tile import TileContext
    from concourse.bass import ts

    f32 = mybir.dt.float32
    f16 = mybir.dt.float16
    f8 = mybir.dt.float8e4
    AF = mybir.ActivationFunctionType
    DR = mybir.MatmulPerfMode.DoubleRow

    nc = bacc.Bacc()
    fb_d = nc.declare_dram_parameter("fb", [128, FB_W], f16, isOutput=False)
    # partition-major fp8 edge: [p, pair, t, j] flattened to [128, 32768]
    edge_d = nc.declare_dram_parameter("edge", [128, NPAIR * 2 * M], f8, isOutput=False)
    bb_d = nc.declare_dram_parameter("bb", [128, BB_W], f32, isOutput=False)
    outT_d = nc.declare_dram_parameter("outT", [D_HID, M], f16, isOutput=True)

    with TileContext(nc) as tc:
        with (
            tc.tile_pool(name="const", bufs=1) as cpool,
            tc.tile_pool(name="acts", bufs=1) as apool,
            tc.tile_pool(name="edgep", bufs=NPAIR) as epool,
            tc.tile_pool(name="pout", bufs=1, space="PSUM") as pout_pool,
            tc.tile_pool(name="pg", bufs=2, space="PSUM") as pg_pool,
            tc.tile_pool(name="pwork", bufs=2, space="PSUM") as pwork_pool,
        ):
            fb = cpool.tile([128, FB_W], f16, name="fb")
            bb = cpool.tile([128, BB_W], f32, name="bb")

            # input DMAs on one queue, small-first, then the edge stream
            nc.sync.dma_start(out=fb, in_=fb_d[:])
            nc.sync.dma_start(out=bb, in_=bb_d[:])
            eds = [epool.tile([128, 2, M], f8, tag="e", name=f"ed{p}") for p in range(NPAIR)]
            for p in range(NPAIR):
                nc.sync.dma_start(out=eds[p], in_=edge_d[:, ts(p, 2 * M)])

            xT = fb[:, _F_XT:_F_XT + M // 2]
            w_g2 = fb[0:128, _W_GW2:_W_GW2 + 128]
            w_f2 = fb[0:64, _W_FW2:_W_FW2 + 128]
            gb2b4 = bb[0:128, _B_GB2:_B_GB2 + 512]
            b_f1 = bb[0:64, _B_F1:_B_F1 + 1]
            b_g1 = bb[0:128, _B_G1:_B_G1 + 1]
            b_f2 = bb[0:128, _B_F2:_B_F2 + 1]

            h1f = apool.tile([D_IN, M], f16, name="h1f")
            h1g = apool.tile([D_HID, M], f16, name="h1g")
            gx = apool.tile([128, NT, 128], f8, name="gx")  # tile i: [t, h] node-major
            outT = apool.tile([128, M], f16, name="outT")
            pout = pout_pool.tile([128, M], f32, name="pout")

            # warm the ACT function table during the preamble (hoists the lazy
            # ~1.3us ACT_TABLE_LOAD off the h1g critical path)
            warm = apool.tile([1, 1], f32, name="warm")
            nc.scalar.activation(warm, bb[0:1, 0:1], AF.Relu, bias=0.0)

            # token-chunk c (tokens 512c..512c+512) reads
            # xT[64a : 64a+64, 512c2 : 512c2+512] with (a, c2) = divmod(c, 2)
            def tok_slice(c):
                a, c2 = divmod(c, 2)
                return slice(64 * a, 64 * a + 64), slice(
                    _F_XT + 512 * c2, _F_XT + 512 * c2 + 512
                )

            def h1g_chunk(c):
                rows, cols = tok_slice(c)
                w_g1 = fb[rows, _W_WG1:_W_WG1 + 128]
                psg = pg_pool.tile([128, 512], f32, tag="g", name="psg")
                nc.tensor.matmul(psg, w_g1, fb[rows, cols], start=True, stop=True)
                nc.scalar.activation(h1g[:, ts(c, 512)], psg, AF.Relu, bias=b_g1)

            def h1f_chunk(c):
                rows, cols = tok_slice(c)
                w_f1 = fb[rows, _W_FW1:_W_FW1 + 64]
                psf = pwork_pool.tile([64, 512], f32, tag="w", name="psf")
                nc.tensor.matmul(psf, w_f1, fb[rows, cols], start=True, stop=True)
                nc.scalar.activation(h1f[:, ts(c, 512)], psf, AF.Relu, bias=b_f1)

            def gx_chunk(c):
                # gx tiles 4c..4c+3 (node-major [t, h]) batched: 4 matmuls into
                # one PSUM bank, one DVE bias-add with fp8 downcast on write
                psx = pwork_pool.tile([128, 512], f32, tag="w", name="psx")
                for k in range(4):
                    i = 4 * c + k
                    nc.tensor.matmul(
                        psx[:, ts(k, 128)], h1g[:, ts(i, 128)], w_g2,
                        start=True, stop=True,
                    )
                nc.vector.tensor_add(gx[:, 4 * c:4 * c + 4, :], psx, gb2b4)

            # ---- dense MLP phase (PE stays busy; edge DMAs stream behind) ----
            h1g_chunk(0)
            gx_chunk(0)
            h1g_chunk(1)
            gx_chunk(1)
            h1f_chunk(0)
            h1f_chunk(1)
            h1g_chunk(2)
            gx_chunk(2)
            h1f_chunk(2)
            h1f_chunk(3)
            h1g_chunk(3)
            gx_chunk(3)
            # self-dynamics opens each output chunk's PSUM accumulation group
            for c in range(NCH):
                nc.tensor.matmul(
                    pout[:, ts(c, 512)], w_f2, h1f[:, ts(c, 512)],
                    start=True, stop=False,
                )

            # ---- gapless DoubleRow edge stream, pair-major ----
            for p in range(NPAIR):
                lhsT = gx[:, 2 * p:2 * p + 2, :]
                for c in range(NCH):
                    nc.tensor.matmul(
                        pout[:, ts(c, 512)], lhsT, eds[p][:, :, ts(c, 512)],
                        start=False, stop=(p == NPAIR - 1),
                        perf_mode=DR,
                    )
                    if p == NPAIR - 1:
                        # chunk c complete: bias-add + downcast immediately
                        src = pout[:, ts(c, 512)]
                        dst = outT[:, ts(c, 512)]
                        if c % 2 == 0:
                            nc.scalar.activation(dst, src, AF.Identity, bias=b_f2)
                        else:
                            nc.vector.tensor_scalar_add(dst, src, b_f2)
                        if c == 1:
                            nc.gpsimd.dma_start(
                                out=outT_d[:, 0:1024], in_=outT[:, 0:1024]
                            )
                        elif c == 3:
                            nc.gpsimd.dma_start(
                                out=outT_d[:, 1024:2048], in_=outT[:, 1024:2048]
                            )
    nc.compile()
    return nc


def _get_nc():
    if "nc" not in _NC_CACHE:
        _NC_CACHE["nc"] = _build()
    return _NC_CACHE["nc"]


def _prep_in_maps(inputs):
    import ml_dtypes

    x = np.asarray(inputs["x"], dtype=np.float32)
    edge = np.asarray(inputs["edge"], dtype=np.float32)
    f_w1 = np.asarray(inputs["f_w1"], dtype=np.float32)
    f_b1 = np.asarray(inputs["f_b1"], dtype=np.float32)
    f_w2 = np.asarray(inputs["f_w2"], dtype=np.float32)
    f_b2 = np.asarray(inputs["f_b2"], dtype=np.float32)
    g_w1 = np.asarray(inputs["g_w1"], dtype=np.float32)
    g_b1 = np.asarray(inputs["g_b1"], dtype=np.float32)
    g_w2 = np.asarray(inputs["g_w2"], dtype=np.float32)
    g_b2 = np.asarray(inputs["g_b2"], dtype=np.float32)

    # cat(x, x) @ g_w1 == x @ (g_w1[:64] + g_w1[64:])
    wg1 = g_w1[:D_IN] + g_w1[D_IN:]

    # x[n].T packed [128, 1024]: xT2[64a + k, t] = x[n, 1024a + t, k]
    xT = np.transpose(x, (0, 2, 1)).astype(np.float16)       # [8, 64, 2048]
    xT2 = np.concatenate([xT[:, :, :1024], xT[:, :, 1024:]], axis=1)  # [8, 128, 1024]

    fb = np.zeros((N_BATCH, 128, FB_W), dtype=np.float16)
    fb[:, :, _F_XT:_F_XT + M // 2] = xT2
    for r in (slice(0, 64), slice(64, 128)):  # duplicate for partition-64 rhs
        fb[:, r, _W_FW1:_W_FW1 + 64] = f_w1.astype(np.float16)
        fb[:, r, _W_FW2:_W_FW2 + 128] = f_w2.astype(np.float16)
        fb[:, r, _W_WG1:_W_WG1 + 128] = wg1.astype(np.float16)
    fb[:, 0:128, _W_GW2:_W_GW2 + 128] = g_w2.astype(np.float16)
    fb = np.ascontiguousarray(fb)

    bb = np.zeros((128, BB_W), dtype=np.float32)
    bb[0:128, _B_GB2:_B_GB2 + 512] = np.tile(g_b2[None, :], (128, 4))
    bb[0:64, _B_F1] = f_b1
    bb[0:128, _B_G1] = g_b1
    bb[0:128, _B_F2] = f_b2

    # edge fp8 partition-major: edge_r[n][p, pair, t, j] = edge8[n, 128*(2*pair+t)+p, j]
    edge8 = edge.astype(ml_dtypes.float8_e4m3)               # [8, 2048, 2048]
    edge_r = np.ascontiguousarray(
        edge8.reshape(N_BATCH, NPAIR, 2, 128, M).transpose(0, 3, 1, 2, 4)
    ).reshape(N_BATCH, 128, NPAIR * 2 * M)

    in_maps = [
        {
            "fb": fb[n],
            "edge": edge_r[n],
            "bb": bb,
        }
        for n in range(N_BATCH)
    ]
    return in_maps


def run(inputs, trace=False, **kw):
    """Run on 8 cores; returns (out [8, 2048, 128] fp32, BassKernelResults)."""
    from concourse.bass_utils import run_bass_kernel_spmd

    nc = _get_nc()
    in_maps = _prep_in_maps(inputs)
    res = run_bass_kernel_spmd(nc, in_maps, list(range(NCORES)), trace=trace, **kw)
    outT = np.stack([np.asarray(res.results[n]["outT"]) for n in range(N_BATCH)])
    out = np.ascontiguousarray(np.transpose(outT, (0, 2, 1)))  # [8, 2048, 128]
    return out.astype(np.float32), res


def kernel(**inputs):
    out, _ = run(inputs, trace=False)
    return out


# revision 5
# speedup vs baseline: 1.2251x; 1.1329x over previous
"""Trainium2 Bass kernel for nn_Node_GCN: out[n] = f(x[n]) + edge[n]^T @ g(cat(x,x)[n]).

Sharding: data-parallel over the batch dim N=8, one batch per NeuronCore.

v3: edge is carried in fp8(e4m3) — halves the dominant HBM stream vs fp16 —
and gx is quantized to fp8 so the 2048x2048x128 edge contraction runs in
MatmulPerfMode.DoubleRow (2 K-tiles per matmul, 216ns/512-col steady-state).
Quantization error lands at rel_l2 ~1.8e-3 (vs 2e-2 budget): edge in [0,1)
keeps e4m3's ulp at 2^-4 scale and the 2048-term reduction averages the
noise down.

All input DMAs ride ONE queue (Sync) ordered small-first — xT+wb packed as
a single fp16 blob, then biases, then 8 pair-granular edge DMAs — so the
MLP inputs land in ~2us and the edge stream follows back-to-back. Edge is
host-reordered partition-major: each pair DMA is 128 descriptors x 4KB
contiguous. The outT store is split in two posts on the idle GpSimd queue,
overlapping the second half's bias-add with the first half's writeback.

The device computes outT[n] = [h, j] fp16; the host transposes to [j, h]
and widens to fp32 while unsharding.
"""

import numpy as np

D_IN = 64
D_HID = 128
M = 2048          # nodes per batch
N_BATCH = 8
NCORES = 8

NT = M // 128     # 16 sender k-tiles
NPAIR = NT // 2   # 8 DoubleRow pairs = 8 edge DMAs
NCH = M // 512    # 4 output chunks of 512

# fp16 blob [128, FB_W]: xT [128, 1024] then weights
_F_XT = 0
_W_FW1 = 1024       # f_w1 [64, 64]
_W_FW2 = 1088       # f_w2 [64, 128]
_W_WG1 = 1216       # wg1  [64, 128]  (= g_w1[:64] + g_w1[64:])
_W_GW2 = 1344       # g_w2 [128, 128]
FB_W = 1472

# fp32 bias blob [128, BB_W]
_B_GB2 = 0          # g_b2 broadcast rows, tiled 4x along free dim [128, 512]
_B_F1 = 512         # f_b1 [64, 1]
_B_G1 = 513         # g_b1 [128, 1]
_B_F2 = 514         # f_b2 [128, 1]
BB_W = 515

_NC_CACHE = {}


def _build():
    import concourse.bacc as bacc
    import concourse.mybir as mybir
    from concourse.tile import TileContext
    from concourse.bass import ts

    f32 = mybir.dt.float32
    f16 = mybir.dt.float16
    f8 = mybir.dt.float8e4
    AF = mybir.ActivationFunctionType
    DR = mybir.MatmulPerfMode.DoubleRow

    nc = bacc.Bacc()
    fb_d = nc.declare_dram_parameter("fb", [128, FB_W], f16, isOutput=False)
    # partition-major fp8 edge: [p, pair, t, j] flattened to [128, 32768]
    edge_d = nc.declare_dram_parameter("edge", [128, NPAIR * 2 * M], f8, isOutput=False)
    bb_d = nc.declare_dram_parameter("bb", [128, BB_W], f32, isOutput=False)
    outT_d = nc.declare_dram_parameter("outT", [D_HID, M], f16, isOutput=True)

    with TileContext(nc) as tc:
        with (
            tc.tile_pool(name="const", bufs=1) as cpool,
            tc.tile_pool(name="acts", bufs=1) as apool,
            tc.tile_pool(name="edgep", bufs=NPAIR) as epool,
            tc.tile_pool(name="pout", bufs=1, space="PSUM") as pout_pool,
            tc.tile_pool(name="pg", bufs=2, space="PSUM") as pg_pool,
            tc.tile_pool(name="pwork", bufs=2, space="PSUM") as pwork_pool,
        ):
            fb = cpool.tile([128, FB_W], f16, name="fb")
            bb = cpool.tile([128, BB_W], f32, name="bb")

            # input DMAs on one queue, small-first, then the edge stream
            nc.sync.dma_start(out=fb, in_=fb_d[:])
            nc.sync.dma_start(out=bb, in_=bb_d[:])
            eds = [epool.tile([128, 2, M], f8, tag="e", name=f"ed{p}") for p in range(NPAIR)]
            for p in range(NPAIR):
                nc.sync.dma_start(out=eds[p], in_=edge_d[:, ts(p, 2 * M)])

            xT = fb[:, _F_XT:_F_XT + M // 2]
            w_g2 = fb[0:128, _W_GW2:_W_GW2 + 128]
            w_f2 = fb[0:64, _W_FW2:_W_FW2 + 128]
            gb2b4 = bb[0:128, _B_GB2:_B_GB2 + 512]
            b_f1 = bb[0:64, _B_F1:_B_F1 + 1]
            b_g1 = bb[0:128, _B_G1:_B_G1 + 1]
            b_f2 = bb[0:128, _B_F2:_B_F2 + 1]

            h1f = apool.tile([D_IN, M], f16, name="h1f")
            h1g = apool.tile([D_HID, M], f16, name="h1g")
            gx = apool.tile([128, NT, 128], f8, name="gx")  # tile i: [t, h] node-major
            # per-chunk tiles: separate dep tracking so chunk c's bias/store
            # never serializes against chunk c+1's matmuls (tile-granular deps)
            outTs = [apool.tile([128, 512], f16, name=f"outT{c}") for c in range(NCH)]
            pouts = [pout_pool.tile([128, 512], f32, name=f"pout{c}") for c in range(NCH)]

            # warm the ACT function table during the preamble (hoists the lazy
            # ~1.3us ACT_TABLE_LOAD off the h1g critical path)
            warm = apool.tile([1, 1], f32, name="warm")
            nc.scalar.activation(warm, bb[0:1, 0:1], AF.Relu, bias=0.0)

            # token-chunk c (tokens 512c..512c+512) reads
            # xT[64a : 64a+64, 512c2 : 512c2+512] with (a, c2) = divmod(c, 2)
            def tok_slice(c):
                a, c2 = divmod(c, 2)
                return slice(64 * a, 64 * a + 64), slice(
                    _F_XT + 512 * c2, _F_XT + 512 * c2 + 512
                )

            def h1g_chunk(c):
                rows, cols = tok_slice(c)
                w_g1 = fb[rows, _W_WG1:_W_WG1 + 128]
                psg = pg_pool.tile([128, 512], f32, tag="g", name="psg")
                nc.tensor.matmul(psg, w_g1, fb[rows, cols], start=True, stop=True)
                nc.scalar.activation(h1g[:, ts(c, 512)], psg, AF.Relu, bias=b_g1)

            def h1f_chunk(c):
                rows, cols = tok_slice(c)
                w_f1 = fb[rows, _W_FW1:_W_FW1 + 64]
                psf = pwork_pool.tile([64, 512], f32, tag="w", name="psf")
                nc.tensor.matmul(psf, w_f1, fb[rows, cols], start=True, stop=True)
                nc.scalar.activation(h1f[:, ts(c, 512)], psf, AF.Relu, bias=b_f1)

            def gx_chunk(c):
                # gx tiles 4c..4c+3 (node-major [t, h]) batched: 4 matmuls into
                # one PSUM bank, one DVE bias-add with fp8 downcast on write
                psx = pwork_pool.tile([128, 512], f32, tag="w", name="psx")
                for k in range(4):
                    i = 4 * c + k
                    nc.tensor.matmul(
                        psx[:, ts(k, 128)], h1g[:, ts(i, 128)], w_g2,
                        start=True, stop=True,
                    )
                nc.vector.tensor_add(gx[:, 4 * c:4 * c + 4, :], psx, gb2b4)

            # ---- MLP phase: interleaved so ACT-latency hides under other
            # Tensor work; edge DMAs stream behind on the same queue ----
            h1f_chunk(0)
            h1f_chunk(1)
            h1g_chunk(0)
            h1f_chunk(2)
            h1f_chunk(3)
            gx_chunk(0)
            h1g_chunk(1)
            # self-dynamics opens each output chunk's PSUM accumulation group
            for c in range(NCH):
                nc.tensor.matmul(
                    pouts[c], w_f2, h1f[:, ts(c, 512)],
                    start=True, stop=False,
                )
            gx_chunk(1)
            h1g_chunk(2)
            gx_chunk(2)
            h1g_chunk(3)
            gx_chunk(3)

            # ---- gapless DoubleRow edge stream, pair-major ----
            for p in range(NPAIR):
                lhsT = gx[:, 2 * p:2 * p + 2, :]
                for c in range(NCH):
                    nc.tensor.matmul(
                        pouts[c], lhsT, eds[p][:, :, ts(c, 512)],
                        start=False, stop=(p == NPAIR - 1),
                        perf_mode=DR,
                    )

            # tail: per-chunk bias-add (alternating ACT/DVE) + per-chunk store
            for c in range(NCH):
                if c % 2 == 0:
                    nc.scalar.activation(outTs[c], pouts[c], AF.Identity, bias=b_f2)
                else:
                    nc.vector.tensor_scalar_add(outTs[c], pouts[c], b_f2)
                nc.gpsimd.dma_start(out=outT_d[:, ts(c, 512)], in_=outTs[c])
    nc.compile()
    return nc


def _get_nc():
    if "nc" not in _NC_CACHE:
        _NC_CACHE["nc"] = _build()
    return _NC_CACHE["nc"]


def _prep_in_maps(inputs):
    import ml_dtypes

    x = np.asarray(inputs["x"], dtype=np.float32)
    edge = np.asarray(inputs["edge"], dtype=np.float32)
    f_w1 = np.asarray(inputs["f_w1"], dtype=np.float32)
    f_b1 = np.asarray(inputs["f_b1"], dtype=np.float32)
    f_w2 = np.asarray(inputs["f_w2"], dtype=np.float32)
    f_b2 = np.asarray(inputs["f_b2"], dtype=np.float32)
    g_w1 = np.asarray(inputs["g_w1"], dtype=np.float32)
    g_b1 = np.asarray(inputs["g_b1"], dtype=np.float32)
    g_w2 = np.asarray(inputs["g_w2"], dtype=np.float32)
    g_b2 = np.asarray(inputs["g_b2"], dtype=np.float32)

    # cat(x, x) @ g_w1 == x @ (g_w1[:64] + g_w1[64:])
    wg1 = g_w1[:D_IN] + g_w1[D_IN:]

    # x[n].T packed [128, 1024]: xT2[64a + k, t] = x[n, 1024a + t, k]
    xT = np.transpose(x, (0, 2, 1)).astype(np.float16)       # [8, 64, 2048]
    xT2 = np.concatenate([xT[:, :, :1024], xT[:, :, 1024:]], axis=1)  # [8, 128, 1024]

    fb = np.zeros((N_BATCH, 128, FB_W), dtype=np.float16)
    fb[:, :, _F_XT:_F_XT + M // 2] = xT2
    for r in (slice(0, 64), slice(64, 128)):  # duplicate for partition-64 rhs
        fb[:, r, _W_FW1:_W_FW1 + 64] = f_w1.astype(np.float16)
        fb[:, r, _W_FW2:_W_FW2 + 128] = f_w2.astype(np.float16)
        fb[:, r, _W_WG1:_W_WG1 + 128] = wg1.astype(np.float16)
    fb[:, 0:128, _W_GW2:_W_GW2 + 128] = g_w2.astype(np.float16)
    fb = np.ascontiguousarray(fb)

    bb = np.zeros((128, BB_W), dtype=np.float32)
    bb[0:128, _B_GB2:_B_GB2 + 512] = np.tile(g_b2[None, :], (128, 4))
    bb[0:64, _B_F1] = f_b1
    bb[0:128, _B_G1] = g_b1
    bb[0:128, _B_F2] = f_b2

    # edge fp8 partition-major: edge_r[n][p, pair, t, j] = edge8[n, 128*(2*pair+t)+p, j]
    edge8 = edge.astype(ml_dtypes.float8_e4m3)               # [8, 2048, 2048]
    edge_r = np.ascontiguousarray(
        edge8.reshape(N_BATCH, NPAIR, 2, 128, M).transpose(0, 3, 1, 2, 4)
    ).reshape(N_BATCH, 128, NPAIR * 2 * M)

    in_maps = [
        {
            "fb": fb[n],
            "edge": edge_r[n],
            "bb": bb,
        }
        for n in range(N_BATCH)
    ]
    return in_maps


def run(inputs, trace=False, **kw):
    """Run on 8 cores; returns (out [8, 2048, 128] fp32, BassKernelResults)."""
    from concourse.bass_utils import run_bass_kernel_spmd

    nc = _get_nc()
    in_maps = _prep_in_maps(inputs)
    res = run_bass_kernel_spmd(nc, in_maps, list(range(NCORES)), trace=trace, **kw)
    outT = np.stack([np.asarray(res.results[n]["outT"]) for n in range(N_BATCH)])
    out = np.ascontiguousarray(np.transpose(outT, (0, 2, 1)))  # [8, 2048, 128]
    return out.astype(np.float32), res


def kernel(**inputs):
    out, _ = run(inputs, trace=False)
    return out


# revision 9
# speedup vs baseline: 1.3101x; 1.0693x over previous
"""Trainium2 Bass kernel for nn_Node_GCN: out[n] = f(x[n]) + edge[n]^T @ g(cat(x,x)[n]).

Sharding: data-parallel over the batch dim N=8, one batch per NeuronCore.

v3: edge is carried in fp8(e4m3) — halves the dominant HBM stream vs fp16 —
and gx is quantized to fp8 so the 2048x2048x128 edge contraction runs in
MatmulPerfMode.DoubleRow (2 K-tiles per matmul, 216ns/512-col steady-state).
Quantization error lands at rel_l2 ~1.8e-3 (vs 2e-2 budget): edge in [0,1)
keeps e4m3's ulp at 2^-4 scale and the 2048-term reduction averages the
noise down.

All input DMAs ride ONE queue (Sync) ordered small-first — xT+wb packed as
a single fp16 blob, then biases, then 8 pair-granular edge DMAs — so the
MLP inputs land in ~2us and the edge stream follows back-to-back. Edge is
host-reordered partition-major: each pair DMA is 128 descriptors x 4KB
contiguous. The outT store is split in two posts on the idle GpSimd queue,
overlapping the second half's bias-add with the first half's writeback.

The device computes outT[n] = [h, j] fp16; the host transposes to [j, h]
and widens to fp32 while unsharding.
"""

import numpy as np

D_IN = 64
D_HID = 128
M = 2048          # nodes per batch
N_BATCH = 8
NCORES = 8

NT = M // 128     # 16 sender k-tiles
NPAIR = NT // 2   # 8 DoubleRow pairs = 8 edge DMAs
NCH = M // 512    # 4 output chunks of 512

# fp16 blob [128, FB_W]: xT [128, 1024] then weights
_F_XT = 0
_W_FW1 = 1024       # f_w1 [64, 64]
_W_FW2 = 1088       # f_w2 [64, 128]
_W_WG1 = 1216       # wg1  [64, 128]  (= g_w1[:64] + g_w1[64:])
_W_GW2 = 1344       # g_w2 [128, 128]
FB_W = 1472

# fp32 bias blob [128, BB_W]
_B_GB2 = 0          # g_b2 broadcast rows, tiled 4x along free dim [128, 512]
_B_F1 = 512         # f_b1 [64, 1]
_B_G1 = 513         # g_b1 [128, 1]
_B_F2 = 514         # f_b2 [128, 1]
BB_W = 515

_NC_CACHE = {}


def _build():
    import concourse.bacc as bacc
    import concourse.mybir as mybir
    from concourse.tile import TileContext
    from concourse.bass import ts

    f32 = mybir.dt.float32
    f16 = mybir.dt.float16
    f8 = mybir.dt.float8e4
    AF = mybir.ActivationFunctionType
    DR = mybir.MatmulPerfMode.DoubleRow

    nc = bacc.Bacc()
    fb_d = nc.declare_dram_parameter("fb", [128, FB_W], f16, isOutput=False)
    # partition-major fp8 edge: [p, pair, t, j] flattened to [128, 32768]
    edge_d = nc.declare_dram_parameter("edge", [128, NPAIR * 2 * M], f8, isOutput=False)
    bb_d = nc.declare_dram_parameter("bb", [128, BB_W], f32, isOutput=False)
    outT_d = nc.declare_dram_parameter("outT", [D_HID, M], f16, isOutput=True)

    with TileContext(nc) as tc:
        with (
            tc.tile_pool(name="const", bufs=1) as cpool,
            tc.tile_pool(name="acts", bufs=1) as apool,
            tc.tile_pool(name="edgep", bufs=NPAIR) as epool,
            tc.tile_pool(name="pout", bufs=1, space="PSUM") as pout_pool,
            tc.tile_pool(name="pg", bufs=2, space="PSUM") as pg_pool,
            tc.tile_pool(name="pwork", bufs=2, space="PSUM") as pwork_pool,
        ):
            fb = cpool.tile([128, FB_W], f16, name="fb")
            bb = cpool.tile([128, BB_W], f32, name="bb")

            # input DMAs on one queue, small-first, then the edge stream.
            # The last pair arrives as 4 chunk-DMAs so the final DoubleRow +
            # bias + store chain hangs off 0.13MB instead of 0.52MB.
            nc.sync.dma_start(out=fb, in_=fb_d[:])
            nc.sync.dma_start(out=bb, in_=bb_d[:])
            eds = [
                epool.tile([128, 2, M], f8, tag="e", name=f"ed{p}")
                for p in range(NPAIR - 1)
            ]
            for p in range(NPAIR - 1):
                nc.sync.dma_start(out=eds[p], in_=edge_d[:, ts(p, 2 * M)])
            ed7 = [
                epool.tile([128, 2, 512], f8, tag="e", name=f"ed7c{c}")
                for c in range(NCH)
            ]
            for c in range(NCH):
                nc.sync.dma_start(
                    out=ed7[c],
                    in_=edge_d[:, (NPAIR - 1) * 2 * M + c * 1024:(NPAIR - 1) * 2 * M + (c + 1) * 1024],
                )

            xT = fb[:, _F_XT:_F_XT + M // 2]
            w_g2 = fb[0:128, _W_GW2:_W_GW2 + 128]
            w_f2 = fb[0:64, _W_FW2:_W_FW2 + 128]
            gb2b4 = bb[0:128, _B_GB2:_B_GB2 + 512]
            b_f1 = bb[0:64, _B_F1:_B_F1 + 1]
            b_g1 = bb[0:128, _B_G1:_B_G1 + 1]
            b_f2 = bb[0:128, _B_F2:_B_F2 + 1]

            h1f = apool.tile([D_IN, M], f16, name="h1f")
            h1g = apool.tile([D_HID, M], f16, name="h1g")
            gx = apool.tile([128, NT, 128], f8, name="gx")  # tile i: [t, h] node-major
            # per-chunk tiles: separate dep tracking so chunk c's bias/store
            # never serializes against chunk c+1's matmuls (tile-granular deps)
            outTs = [apool.tile([128, 512], f16, name=f"outT{c}") for c in range(NCH)]
            pouts = [pout_pool.tile([128, 512], f32, name=f"pout{c}") for c in range(NCH)]

            # warm the ACT function table during the preamble (hoists the lazy
            # ~1.3us ACT_TABLE_LOAD off the h1g critical path)
            warm = apool.tile([1, 1], f32, name="warm")
            nc.scalar.activation(warm, bb[0:1, 0:1], AF.Relu, bias=0.0)

            # PE warm-up: the tensor engine clock-gates to ~half speed unless
            # continuously busy for ~3us. Real work can't start until the fb
            # DMA lands (~10.5us); fill the dead window with dummy matmuls on
            # a memset scratch tile so the first real matmul runs at full
            # clock instead of re-ramping.
            scratch = apool.tile([128, 256], f16, name="scratch")
            nc.gpsimd.memset(scratch, 0)
            for _ in range(20):
                psd = pg_pool.tile([128, 256], f32, tag="g", name="psd")
                nc.tensor.matmul(psd, scratch[:, 0:128], scratch, start=True, stop=True)

            # token-chunk c (tokens 512c..512c+512) reads
            # xT[64a : 64a+64, 512c2 : 512c2+512] with (a, c2) = divmod(c, 2)
            def tok_slice(c):
                a, c2 = divmod(c, 2)
                return slice(64 * a, 64 * a + 64), slice(
                    _F_XT + 512 * c2, _F_XT + 512 * c2 + 512
                )

            def h1g_chunk(c):
                rows, cols = tok_slice(c)
                w_g1 = fb[rows, _W_WG1:_W_WG1 + 128]
                psg = pg_pool.tile([128, 512], f32, tag="g", name="psg")
                nc.tensor.matmul(psg, w_g1, fb[rows, cols], start=True, stop=True)
                nc.scalar.activation(h1g[:, ts(c, 512)], psg, AF.Relu, bias=b_g1)

            def h1f_chunk(c):
                rows, cols = tok_slice(c)
                w_f1 = fb[rows, _W_FW1:_W_FW1 + 64]
                psf = pwork_pool.tile([64, 512], f32, tag="w", name="psf")
                nc.tensor.matmul(psf, w_f1, fb[rows, cols], start=True, stop=True)
                nc.scalar.activation(h1f[:, ts(c, 512)], psf, AF.Relu, bias=b_f1)

            def gx_chunk(c):
                # gx tiles 4c..4c+3 (node-major [t, h]) batched: 4 matmuls into
                # one PSUM bank, one DVE bias-add with fp8 downcast on write
                psx = pwork_pool.tile([128, 512], f32, tag="w", name="psx")
                for k in range(4):
                    i = 4 * c + k
                    nc.tensor.matmul(
                        psx[:, ts(k, 128)], h1g[:, ts(i, 128)], w_g2,
                        start=True, stop=True,
                    )
                nc.vector.tensor_add(gx[:, 4 * c:4 * c + 4, :], psx, gb2b4)

            # ---- MLP phase: interleaved so ACT-latency hides under other
            # Tensor work; edge DMAs stream behind on the same queue ----
            h1f_chunk(0)
            h1f_chunk(1)
            h1g_chunk(0)
            h1f_chunk(2)
            h1f_chunk(3)
            gx_chunk(0)
            h1g_chunk(1)
            # self-dynamics opens each output chunk's PSUM accumulation group
            for c in range(NCH):
                nc.tensor.matmul(
                    pouts[c], w_f2, h1f[:, ts(c, 512)],
                    start=True, stop=False,
                )
            gx_chunk(1)
            h1g_chunk(2)
            gx_chunk(2)
            h1g_chunk(3)
            gx_chunk(3)

            # ---- gapless DoubleRow edge stream, pair-major ----
            for p in range(NPAIR):
                lhsT = gx[:, 2 * p:2 * p + 2, :]
                for c in range(NCH):
                    rhs = ed7[c] if p == NPAIR - 1 else eds[p][:, :, ts(c, 512)]
                    nc.tensor.matmul(
                        pouts[c], lhsT, rhs,
                        start=False, stop=(p == NPAIR - 1),
                        perf_mode=DR,
                    )

            # tail: per-chunk bias-add (alternating ACT/DVE) + per-chunk store
            for c in range(NCH):
                if c % 2 == 0:
                    nc.scalar.activation(outTs[c], pouts[c], AF.Identity, bias=b_f2)
                else:
                    nc.vector.tensor_scalar_add(outTs[c], pouts[c], b_f2)
                nc.gpsimd.dma_start(out=outT_d[:, ts(c, 512)], in_=outTs[c])
    nc.compile()
    return nc


def _get_nc():
    if "nc" not in _NC_CACHE:
        _NC_CACHE["nc"] = _build()
    return _NC_CACHE["nc"]


def _prep_in_maps(inputs):
    import ml_dtypes

    x = np.asarray(inputs["x"], dtype=np.float32)
    edge = np.asarray(inputs["edge"], dtype=np.float32)
    f_w1 = np.asarray(inputs["f_w1"], dtype=np.float32)
    f_b1 = np.asarray(inputs["f_b1"], dtype=np.float32)
    f_w2 = np.asarray(inputs["f_w2"], dtype=np.float32)
    f_b2 = np.asarray(inputs["f_b2"], dtype=np.float32)
    g_w1 = np.asarray(inputs["g_w1"], dtype=np.float32)
    g_b1 = np.asarray(inputs["g_b1"], dtype=np.float32)
    g_w2 = np.asarray(inputs["g_w2"], dtype=np.float32)
    g_b2 = np.asarray(inputs["g_b2"], dtype=np.float32)

    # cat(x, x) @ g_w1 == x @ (g_w1[:64] + g_w1[64:])
    wg1 = g_w1[:D_IN] + g_w1[D_IN:]

    # x[n].T packed [128, 1024]: xT2[64a + k, t] = x[n, 1024a + t, k]
    xT = np.transpose(x, (0, 2, 1)).astype(np.float16)       # [8, 64, 2048]
    xT2 = np.concatenate([xT[:, :, :1024], xT[:, :, 1024:]], axis=1)  # [8, 128, 1024]

    fb = np.zeros((N_BATCH, 128, FB_W), dtype=np.float16)
    fb[:, :, _F_XT:_F_XT + M // 2] = xT2
    for r in (slice(0, 64), slice(64, 128)):  # duplicate for partition-64 rhs
        fb[:, r, _W_FW1:_W_FW1 + 64] = f_w1.astype(np.float16)
        fb[:, r, _W_FW2:_W_FW2 + 128] = f_w2.astype(np.float16)
        fb[:, r, _W_WG1:_W_WG1 + 128] = wg1.astype(np.float16)
    fb[:, 0:128, _W_GW2:_W_GW2 + 128] = g_w2.astype(np.float16)
    fb = np.ascontiguousarray(fb)

    bb = np.zeros((128, BB_W), dtype=np.float32)
    bb[0:128, _B_GB2:_B_GB2 + 512] = np.tile(g_b2[None, :], (128, 4))
    bb[0:64, _B_F1] = f_b1
    bb[0:128, _B_G1] = g_b1
    bb[0:128, _B_F2] = f_b2

    # edge fp8 partition-major: edge_r[n][p, pair, t, j] = edge8[n, 128*(2*pair+t)+p, j]
    # except pair 7, which is chunk-major [p, c, t, j'] for the 4 tail DMAs
    edge8 = edge.astype(ml_dtypes.float8_e4m3)               # [8, 2048, 2048]
    edge_r = (
        edge8.reshape(N_BATCH, NPAIR, 2, 128, M)
        .transpose(0, 3, 1, 2, 4)
        .reshape(N_BATCH, 128, NPAIR, 2 * M)
        .copy()
    )
    p7 = (
        edge8[:, (NPAIR - 1) * 256:, :]                      # [8, 256, 2048]
        .reshape(N_BATCH, 2, 128, NCH, 512)
        .transpose(0, 2, 3, 1, 4)                            # [8, 128, c, t, 512]
        .reshape(N_BATCH, 128, 2 * M)
    )
    edge_r[:, :, NPAIR - 1, :] = p7
    edge_r = np.ascontiguousarray(edge_r).reshape(N_BATCH, 128, NPAIR * 2 * M)

    in_maps = [
        {
            "fb": fb[n],
            "edge": edge_r[n],
            "bb": bb,
        }
        for n in range(N_BATCH)
    ]
    return in_maps


def run(inputs, trace=False, **kw):
    """Run on 8 cores; returns (out [8, 2048, 128] fp32, BassKernelResults)."""
    from concourse.bass_utils import run_bass_kernel_spmd

    nc = _get_nc()
    in_maps = _prep_in_maps(inputs)
    res = run_bass_kernel_spmd(nc, in_maps, list(range(NCORES)), trace=trace, **kw)
    outT = np.stack([np.asarray(res.results[n]["outT"]) for n in range(N_BATCH)])
    out = np.ascontiguousarray(np.transpose(outT, (0, 2, 1)))  # [8, 2048, 128]
    return out.astype(np.float32), res


def kernel(**inputs):
    out, _ = run(inputs, trace=False)
    return out
